# revision 5
# baseline (speedup 1.0000x reference)
"""Trainium2 Bass kernel for the Performer-style random-feature map:

    out[n, s] = exp(-||x_n||^2 / 2) * S^{-1/2} * exp((x @ W.T)[n, s] + b[s])
              = exp((x @ W.T)[n, s] + b[s] - 0.5*||x_n||^2 - 0.5*ln(S))

Sharding: data-parallel over the N (row) axis across 8 NeuronCores; W and b
replicated.  Each core computes a [2048, 2048] output block.  Pure SPMD, no
collectives.

Per-core structure (all sizes hardcoded for N=16384, D=1024, S=2048):
  - x^T  [1024, 2048] f32 and W^T [1024, 2048] f32 live in SBUF as 8 k-strips
    of [128, *]; the matmul contracts over d on partitions (float32r at
    1 cyc/row for free dims >= 256).
  - natural-layout x rows stream in per 128-row block; one fused DVE
    tensor_tensor_reduce computes bias_n = -0.5*||x_n||^2 - 0.5*ln(S).
  - per output tile: 8 accumulating matmuls -> PSUM, DVE adds broadcast b,
    ACT computes exp(psum + bias_n) -> SBUF, DMA out.
"""

import sys
from contextlib import ExitStack

if "/opt/trn_rl_repo" not in sys.path:
    sys.path.insert(0, "/opt/trn_rl_repo")

import numpy as np

import concourse.bacc as bacc
import concourse.tile as tile
from concourse import mybir

P = 128          # SBUF partitions
N_FULL = 16384   # total rows
D_FULL = 1024    # contraction dim
S_FULL = 2048    # output features
N_CORES = 8
NC_FULL = N_FULL // N_CORES  # rows per core

F32 = mybir.dt.float32
BF16 = mybir.dt.bfloat16


def build_nc(NCc=NC_FULL, D=D_FULL, S=S_FULL, psum_w=1024,
             mm_n=512, psum_bufs=3):
    """Build the single-core Bass program (same program runs SPMD on 8 cores)."""
    nc = bacc.Bacc("TRN2", target_bir_lowering=False, debug=False)

    xT = nc.dram_tensor("xT", [D, NCc], BF16, kind="ExternalInput").ap()
    xn = nc.dram_tensor("xn", [NCc, D], F32, kind="ExternalInput").ap()
    w = nc.dram_tensor("w", [D, S], BF16, kind="ExternalInput").ap()
    bv = nc.dram_tensor("bias", [S], F32, kind="ExternalInput").ap()
    out = nc.dram_tensor("out", [NCc, S], F32, kind="ExternalOutput").ap()

    KT = D // P            # k tiles (contraction)
    NB = NCc // P          # 128-row output blocks
    NS = min(mm_n, S)      # matmul moving free dim (<= 512 for one PSUM bank)
    S2 = min(psum_w, S)    # psum tile width
    SH = S // S2           # psum tiles per row block
    neg_half_ln_s = float(-0.5 * np.log(S))

    with tile.TileContext(nc) as tc, ExitStack() as ctx:
        singles = ctx.enter_context(tc.tile_pool(name="singles", bufs=1))
        w_sb = singles.tile([P, KT, S], BF16)
        x_sb = singles.tile([P, KT, NCc], BF16)
        b_bc = singles.tile([P, S], F32)
        bias_tiles = [
            singles.tile([P, 1], F32, tag=f"bias{nb}", name=f"bias{nb}")
            for nb in range(NB)
        ]

        for k in range(KT):
            nc.sync.dma_start(w_sb[:, k, :], w[k * P:(k + 1) * P, :])
            nc.sync.dma_start(x_sb[:, k, :], xT[k * P:(k + 1) * P, :])
        import concourse.bass as bass
        bv_bcast = bass.AP(tensor=bv.tensor, offset=bv.offset,
                           ap=[[0, P]] + list(bv.ap))
        nc.sync.dma_start(b_bc, bv_bcast)

        # bias_n = -0.5 * sum_d x[n,d]^2 - 0.5*ln(S), one [P,1] tile per block
        xn_pool = ctx.enter_context(tc.tile_pool(name="xnp", bufs=3))
        sq_pool = ctx.enter_context(tc.tile_pool(name="sqp", bufs=2))
        r_pool = ctx.enter_context(tc.tile_pool(name="rp", bufs=4))
        for nb in range(NB):
            xt = xn_pool.tile([P, D], F32)
            nc.sync.dma_start(xt, xn[nb * P:(nb + 1) * P, :])
            sq = sq_pool.tile([P, D], F32)
            nc.vector.tensor_mul(sq, xt, xt)
            r_raw = r_pool.tile([P, 1], F32)
            nc.vector.tensor_reduce(
                r_raw, sq, axis=mybir.AxisListType.X, op=mybir.AluOpType.add)
            nc.vector.tensor_scalar(
                out=bias_tiles[nb], in0=r_raw,
                scalar1=-0.5, scalar2=neg_half_ln_s,
                op0=mybir.AluOpType.mult, op1=mybir.AluOpType.add)

        psum_pool = ctx.enter_context(
            tc.tile_pool(name="psum", bufs=psum_bufs, space="PSUM"))
        out_pool = ctx.enter_context(tc.tile_pool(name="osb", bufs=2))
        for nb in range(NB):
            o_sb = out_pool.tile([P, S], F32)
            for h in range(SH):
                ps = psum_pool.tile([P, S2], F32)
                for c in range(S2 // NS):
                    col0 = h * S2 + c * NS
                    for k in range(KT):
                        nc.tensor.matmul(
                            ps[:, c * NS:(c + 1) * NS],
                            lhsT=x_sb[:, k, nb * P:(nb + 1) * P],
                            rhs=w_sb[:, k, col0:col0 + NS],
                            start=(k == 0),
                            stop=(k == KT - 1),
                        )
                nc.vector.tensor_add(ps, ps, b_bc[:, h * S2:(h + 1) * S2])
                nc.scalar.activation(
                    o_sb[:, h * S2:(h + 1) * S2],
                    ps,
                    func=mybir.ActivationFunctionType.Exp,
                    bias=bias_tiles[nb],
                    scale=1.0,
                )
            nc.sync.dma_start(out[nb * P:(nb + 1) * P, :], o_sb)

    nc.compile()
    return nc


_NC_CACHE = {}


def _get_nc(**kwargs):
    key = tuple(sorted(kwargs.items()))
    if key not in _NC_CACHE:
        _NC_CACHE[key] = build_nc(**kwargs)
    return _NC_CACHE[key]


def make_in_maps(x, W, b):
    import ml_dtypes
    bf16 = ml_dtypes.bfloat16
    wT = np.ascontiguousarray(W.T.astype(bf16))
    b = np.ascontiguousarray(b.astype(np.float32))
    in_maps = []
    for i in range(N_CORES):
        xs = np.ascontiguousarray(x[i * NC_FULL:(i + 1) * NC_FULL].astype(np.float32))
        in_maps.append({
            "xT": np.ascontiguousarray(xs.T.astype(bf16)),
            "xn": xs,
            "w": wT,
            "bias": b,
        })
    return in_maps


def run_hw(x, W, b, trace=False, **build_kwargs):
    """Run on 8 NeuronCores; returns (out [N, S] f32, BassKernelResults)."""
    from concourse.bass_utils import run_bass_kernel_spmd
    from concourse.bass_interp import get_hw_module

    nc = _get_nc(**build_kwargs)
    in_maps = make_in_maps(x, W, b)
    old_m = nc.m
    nc.m = get_hw_module(nc.m)
    try:
        res = run_bass_kernel_spmd(
            nc, in_maps, core_ids=list(range(N_CORES)), trace=trace)
    finally:
        nc.m = old_m
    out = np.concatenate(
        [res.results[i]["out"] for i in range(N_CORES)], axis=0)
    return out.astype(np.float32), res


def kernel(x, W, b):
    out, _ = run_hw(x, W, b, trace=False)
    return out


# revision 6
# speedup vs baseline: 1.1215x; 1.1215x over previous
"""Trainium2 Bass kernel for the Performer-style random-feature map:

    out[n, s] = exp(-||x_n||^2 / 2) * S^{-1/2} * exp((x @ W.T)[n, s] + b[s])
              = exp((x @ W.T)[n, s] - 0.5*||x_n||^2 - 0.5*ln(S)) * exp(b[s])

Sharding: data-parallel over the N (row) axis across 8 NeuronCores; W and b
replicated.  Each core computes a [2048, 2048] output block.  Pure SPMD, no
collectives.

Per-core structure (sizes hardcoded for N=16384, D=1024, S=2048):
  - x^T and W^T live in SBUF as bf16 k-strips of [128, *] (one tile per
    strip so matmuls only wait on the strip they need); the matmul
    contracts over d on partitions.
  - natural-layout x rows stream in per 128-row block; DVE computes
    bias_n = -0.5*||x_n||^2 - 0.5*ln(S) as a per-partition scalar.
  - per [128, 1024] PSUM group: 16 accumulating matmuls -> ACT exp(psum +
    bias_n) -> GpSimd multiply by exp(b) broadcast -> DMA out.
"""

import sys
from contextlib import ExitStack

if "/opt/trn_rl_repo" not in sys.path:
    sys.path.insert(0, "/opt/trn_rl_repo")

import numpy as np

import concourse.bacc as bacc
import concourse.bass as bass
import concourse.tile as tile
from concourse import mybir

P = 128          # SBUF partitions
N_FULL = 16384   # total rows
D_FULL = 1024    # contraction dim
S_FULL = 2048    # output features
N_CORES = 8
NC_FULL = N_FULL // N_CORES  # rows per core

F32 = mybir.dt.float32
BF16 = mybir.dt.bfloat16


def build_nc(NCc=NC_FULL, D=D_FULL, S=S_FULL, psum_w=1024,
             mm_n=512, psum_bufs=4, eb_engine="gpsimd"):
    """Build the single-core Bass program (same program runs SPMD on 8 cores)."""
    nc = bacc.Bacc("TRN2", target_bir_lowering=False, debug=False)

    xT = nc.dram_tensor("xT", [D, NCc], BF16, kind="ExternalInput").ap()
    xn = nc.dram_tensor("xn", [NCc, D], F32, kind="ExternalInput").ap()
    w = nc.dram_tensor("w", [D, S], BF16, kind="ExternalInput").ap()
    bv = nc.dram_tensor("bias", [S], F32, kind="ExternalInput").ap()
    out = nc.dram_tensor("out", [NCc, S], F32, kind="ExternalOutput").ap()

    KT = D // P            # k tiles (contraction)
    NB = NCc // P          # 128-row output blocks
    NS = min(mm_n, S)      # matmul moving free dim (<= 512 for one PSUM bank)
    S2 = min(psum_w, S)    # psum tile width
    SH = S // S2           # psum tiles per row block
    neg_half_ln_s = float(-0.5 * np.log(S))

    with tile.TileContext(nc) as tc, ExitStack() as ctx:
        singles = ctx.enter_context(tc.tile_pool(name="singles", bufs=1))
        w_ks = [singles.tile([P, S], BF16, tag=f"w{k}", name=f"w{k}")
                for k in range(KT)]
        x_ks = [singles.tile([P, NCc], BF16, tag=f"x{k}", name=f"x{k}")
                for k in range(KT)]
        b_bc = singles.tile([P, S], F32)
        eb = singles.tile([P, S], F32)
        bias_tiles = [
            singles.tile([P, 1], F32, tag=f"bias{nb}", name=f"bias{nb}")
            for nb in range(NB)
        ]

        # natural-layout x streams on the scalar (qAct) DMA ring; the
        # per-block exp bias must be ready early, so these go first there.
        xn_pool = ctx.enter_context(tc.tile_pool(name="xnp", bufs=3))
        sq_pool = ctx.enter_context(tc.tile_pool(name="sqp", bufs=2))
        r_pool = ctx.enter_context(tc.tile_pool(name="rp", bufs=4))
        for nb in range(NB):
            xt = xn_pool.tile([P, D], F32)
            nc.scalar.dma_start(xt, xn[nb * P:(nb + 1) * P, :])
            sq = sq_pool.tile([P, D], F32)
            nc.vector.tensor_mul(sq, xt, xt)
            r_raw = r_pool.tile([P, 1], F32)
            nc.vector.tensor_reduce(
                r_raw, sq, axis=mybir.AxisListType.X, op=mybir.AluOpType.add)
            nc.vector.tensor_scalar(
                out=bias_tiles[nb], in0=r_raw,
                scalar1=-0.5, scalar2=neg_half_ln_s,
                op0=mybir.AluOpType.mult, op1=mybir.AluOpType.add)

        # k-strips interleaved so strip 0 of both operands lands first
        for k in range(KT):
            nc.sync.dma_start(x_ks[k], xT[k * P:(k + 1) * P, :])
            nc.sync.dma_start(w_ks[k], w[k * P:(k + 1) * P, :])
        bv_bcast = bass.AP(tensor=bv.tensor, offset=bv.offset,
                           ap=[[0, P]] + list(bv.ap))
        nc.sync.dma_start(b_bc, bv_bcast)
        nc.scalar.activation(eb, b_bc, func=mybir.ActivationFunctionType.Exp)

        psum_pool = ctx.enter_context(
            tc.tile_pool(name="psum", bufs=psum_bufs, space="PSUM"))
        tmp_pool = ctx.enter_context(tc.tile_pool(name="tmp", bufs=3))
        out_pool = ctx.enter_context(tc.tile_pool(name="osb", bufs=2))
        for nb in range(NB):
            o_sb = out_pool.tile([P, S], F32)
            for h in range(SH):
                ps = psum_pool.tile([P, S2], F32)
                for c in range(S2 // NS):
                    col0 = h * S2 + c * NS
                    for k in range(KT):
                        nc.tensor.matmul(
                            ps[:, c * NS:(c + 1) * NS],
                            lhsT=x_ks[k][:, nb * P:(nb + 1) * P],
                            rhs=w_ks[k][:, col0:col0 + NS],
                            start=(k == 0),
                            stop=(k == KT - 1),
                        )
                tmp = tmp_pool.tile([P, S2], F32)
                nc.scalar.activation(
                    tmp, ps,
                    func=mybir.ActivationFunctionType.Exp,
                    bias=bias_tiles[nb],
                    scale=1.0,
                )
                hsl = slice(h * S2, (h + 1) * S2)
                if eb_engine == "gpsimd":
                    nc.gpsimd.tensor_mul(o_sb[:, hsl], tmp, eb[:, hsl])
                else:
                    nc.vector.tensor_mul(o_sb[:, hsl], tmp, eb[:, hsl])
            nc.sync.dma_start(out[nb * P:(nb + 1) * P, :], o_sb)

    nc.compile()
    return nc


_NC_CACHE = {}


def _get_nc(**kwargs):
    key = tuple(sorted(kwargs.items()))
    if key not in _NC_CACHE:
        _NC_CACHE[key] = build_nc(**kwargs)
    return _NC_CACHE[key]


def make_in_maps(x, W, b):
    import ml_dtypes
    bf16 = ml_dtypes.bfloat16
    wT = np.ascontiguousarray(W.T.astype(bf16))
    b = np.ascontiguousarray(b.astype(np.float32))
    in_maps = []
    for i in range(N_CORES):
        xs = np.ascontiguousarray(x[i * NC_FULL:(i + 1) * NC_FULL].astype(np.float32))
        in_maps.append({
            "xT": np.ascontiguousarray(xs.T.astype(bf16)),
            "xn": xs,
            "w": wT,
            "bias": b,
        })
    return in_maps


def run_hw(x, W, b, trace=False, **build_kwargs):
    """Run on 8 NeuronCores; returns (out [N, S] f32, BassKernelResults)."""
    from concourse.bass_utils import run_bass_kernel_spmd
    from concourse.bass_interp import get_hw_module

    nc = _get_nc(**build_kwargs)
    in_maps = make_in_maps(x, W, b)
    old_m = nc.m
    nc.m = get_hw_module(nc.m)
    try:
        res = run_bass_kernel_spmd(
            nc, in_maps, core_ids=list(range(N_CORES)), trace=trace)
    finally:
        nc.m = old_m
    out = np.concatenate(
        [res.results[i]["out"] for i in range(N_CORES)], axis=0)
    return out.astype(np.float32), res


def kernel(x, W, b):
    out, _ = run_hw(x, W, b, trace=False)
    return out
